# revision 34
# baseline (speedup 1.0000x reference)
"""Trainium2 Bass kernel for nn_CrossAttention_DenseAVInteractions.

Math: the reference builds a cartesian KV grid kv[b,i,j] = pv[b,i] + pa[b,j]
over (N_v, N_a) and attends 64 queries against all N_v*N_a = 65536 keys.
Because the logits decompose as s[q,(i,j)] = (q.k_v[i]) + (q.k_a[j]), the
softmax over the product grid factorizes exactly:

    p[q,(i,j)] = softmax_i(q.k_v)[q,i] * softmax_j(q.k_a)[q,j]
    out[q]     = softmax_i(q.k_v) @ v_v + softmax_j(q.k_a) @ v_a

so the whole attention reduces to two 256-key attentions per (b, h).

Sharding (8 cores): core c handles batch b = c // 4 and the head pair
(2j, 2j+1) with j = c % 4.  Each core computes its heads' partial output
projection partial = out_heads @ Wproj[:, head_cols].T; the host sums
the 4 partials per batch and adds bproj.

Device-side design (v2 — rebuilt from the first trace rounds):
 - Everything is bf16 (measured end-to-end rel err ~5e-3 vs the 2e-2 gate),
   halving both HBM traffic (1.37 MB/core) and engine copy time.  fp8 was
   measured too lossy (5.5e-2) and is not used.
 - Scores are computed TRANSPOSED from the start: sT[keys, (h,q)] via
   per-head 64-contract matmuls into PE row-groups.  exp(sT) is then
   directly the PV operand (keys on partitions), eliminating all four
   128x128 p-transposes of the v1 kernel.  The softmax denominator comes
   from a ones-matmul (sum over key partitions), the reciprocal is applied
   per-partition on the tiny o[(h,q), ch] tile after PV.
 - V is projected directly into [keys, ch] layout (8 N=128 matmuls per
   side) instead of [ch, keys] + PE transposes.
 - One [128,64] transpose + 2 [64,64] assembly copies rebuild oA[(h,ch), q]
   for the output projection; the tail is split per head so head 0's
   normalize/transpose overlaps head 1's PV.
 - Inputs stream as 10 chunks over both HWDGE queues (sync + scalar), in
   consumption order; a short cold-PE warmup burst fills the DMA dead time.
"""

import os
import sys

import numpy as np

sys.path.insert(0, "/opt/trn_rl_repo")

DIM = 512
H = 8
HD = DIM // H          # 64
B = 2
N_MM = 64
N_A = 256
N_V = 256
SCALE = HD ** -0.5     # 0.125
N_CORES = 8

# pack column offsets (bf16 columns in the [128, 5376] packed input)
O_WQ = 0          # 4 k-tiles x 128
O_XMM = 512       # 4 x 64
O_WKA = 768       # 4 x 128
O_XA = 1280       # 4 x 256
O_WVA = 2304      # 4 x 128
O_WKV = 2816      # 4 x 128
O_XV = 3328       # 4 x 256
O_WVV = 4352      # 4 x 128
O_WPROJ = 4864    # [128ch, 512]
PACK_COLS = 5376

# chunk boundaries (cols) and which engine issues the load; per-engine
# emission order = HW queue FIFO order.  sync carries q + a-side k-path,
# scalar carries a-side v-weights + v-side; the late-needed wvv/wproj ride
# the gpsimd SWDGE so neither HWDGE sequencer pays their issue cost.
CHUNKS = [
    (O_WQ, 768, "sync"),        # wq + xmm  (q2z gates the whole scores chain)
    (O_XA, 1792, "scalar"),     # xa k0,k1
    (O_WKA, 1280, "sync"),      # wka
    (O_WKV, 3328, "scalar"),    # wkv
    (1792, 2304, "sync"),       # xa k2,k3
    (O_XV, 3840, "scalar"),     # xv k0,k1
    (O_WVA, 2816, "sync"),      # wva
    (3840, 4352, "scalar"),     # xv k2,k3
    (O_WPROJ, 5376, "sync"),    # wproj
    (O_WVV, 4864, "scalar"),    # wvv
]

N_WARMUP = 7

_cached = {}


def _build_program():
    import concourse.bacc as bacc
    from concourse import mybir
    from concourse.tile import TileContext

    f32 = mybir.dt.float32
    bf16 = mybir.dt.bfloat16
    nc = bacc.Bacc(name="cross_attn_dense_av2")

    packA = nc.dram_tensor("packA", [128, PACK_COLS], bf16, kind="ExternalInput")
    out_d = nc.dram_tensor("out", [64, 512], bf16, kind="ExternalOutput")
    import ml_dtypes
    ident_d = nc.inline_tensor(
        np.eye(128).astype(ml_dtypes.bfloat16), name="ident128"
    )

    from contextlib import ExitStack

    with TileContext(nc) as tc, ExitStack() as ctx:
        io = ctx.enter_context(tc.tile_pool(name="io", bufs=1))
        work = ctx.enter_context(tc.tile_pool(name="work", bufs=1))
        # PSUM budget is 8 banks, one per pool buffer:
        #   ps_mid (2): warm -> sT_a -> v_a -> sT_v -> v_v rotation
        #   ps_k   (1): kT_a -> kT_v
        #   ps_sm  (3): q -> z_a -> o_a -> z_v -> o_v rotation
        #   ps_f   (1): oT -> f
        ps_mid = ctx.enter_context(tc.tile_pool(name="ps_mid", bufs=2, space="PSUM"))
        ps_k = ctx.enter_context(tc.tile_pool(name="ps_k", bufs=1, space="PSUM"))
        ps_sm = ctx.enter_context(tc.tile_pool(name="ps_sm", bufs=3, space="PSUM"))
        ps_f = ctx.enter_context(tc.tile_pool(name="ps_f", bufs=2, space="PSUM"))

        # ---- loads: chunks in consumption order on both HWDGE queues ----
        ident = io.tile([128, 128], bf16, tag="ident")
        nc.gpsimd.dma_start(out=ident, in_=ident_d[:, :])
        chunk_t = {}
        for lo, hi, eng in CHUNKS:
            t = io.tile([128, hi - lo], bf16, tag=f"c{lo}")
            getattr(nc, eng).dma_start(out=t, in_=packA[:, lo:hi])
            chunk_t[lo] = t

        def col(off, width):
            """AP slice of the packed input at absolute column offset."""
            for lo, hi, _ in CHUNKS:
                if lo <= off and off + width <= hi:
                    return chunk_t[lo][:, off - lo:off - lo + width]
            raise ValueError(f"span {off}:{off + width} crosses chunk boundary")

        # ---- PE warmup: cold matmuls on memset scratch while the first
        #      input chunks are still in flight ----
        warm_sb = io.tile([128, 256], bf16, tag="warm_sb")
        nc.vector.memset(warm_sb, 0.5)
        ones = io.tile([128, 1], bf16, tag="ones")
        nc.vector.memset(ones, 1.0)
        warm_ps = ps_mid.tile([128, 256], f32, tag="mid")
        for w in range(N_WARMUP):
            nc.tensor.matmul(
                warm_ps, warm_sb[:, 0:128], warm_sb,
                start=(w == 0), stop=(w == N_WARMUP - 1),
            )

        def kproj(o_wk, o_x, side, ceng):
            """kT [128(2h,64ch), 256tok] -> SBUF bf16 (half-split copies)"""
            kp = ps_k.tile([128, 256], f32, tag="k")
            for k in range(4):
                nc.tensor.matmul(
                    kp, col(o_wk + 128 * k, 128), col(o_x + 256 * k, 256),
                    start=(k == 0), stop=(k == 3),
                )
            ks = work.tile([128, 256], bf16, tag=f"k_sb{side}")
            for half in range(2):
                hs = slice(128 * half, 128 * half + 128)
                ceng(ks[:, hs], kp[:, hs])
            return ks

        def scores(ks, side):
            """sT [128keys(half on free), 2x128(h,q)]; exp -> SBUF bf16"""
            sp = ps_mid.tile([128, 256], f32, tag="mid")
            for half in range(2):
                nc.tensor.matmul(
                    sp[:, 128 * half:128 * half + 128],
                    ks[:, 128 * half:128 * half + 128],
                    q2z,
                    start=True, stop=True,
                )
            ex = work.tile([128, 256], bf16, tag=f"e_sb{side}")
            nc.scalar.activation(ex, sp, mybir.ActivationFunctionType.Exp)
            return ex

        def vproj(o_wv, o_x, side, eng):
            """v [128keys(half on free), 2x128(2h,64ch)] -> SBUF bf16"""
            vp = ps_mid.tile([128, 256], f32, tag="mid")
            for half in range(2):
                for k in range(4):
                    nc.tensor.matmul(
                        vp[:, 128 * half:128 * half + 128],
                        col(o_x + 256 * k + 128 * half, 128),
                        col(o_wv + 128 * k, 128),
                        start=(k == 0), stop=(k == 3),
                    )
            v = work.tile([128, 256], bf16, tag=f"v_sb{side}")
            for half in range(2):
                eng.tensor_copy(
                    v[:, 128 * half:128 * half + 128],
                    vp[:, 128 * half:128 * half + 128],
                )
            return v

        def zsum(ex, side):
            """z [128(h,q), 1] = sum over keys; reciprocal on DVE -> f32"""
            zp = ps_sm.tile([128, 1], f32, tag="sm")
            for half in range(2):
                nc.tensor.matmul(
                    zp, ex[:, 128 * half:128 * half + 128], ones,
                    start=(half == 0), stop=(half == 1),
                )
            zr = work.tile([128, 1], f32, tag=f"zr{side}")
            nc.vector.reciprocal(zr, zp)
            return zr

        def pv(ex, v, side):
            """o [128(h,q), 64ch] PSUM, per-head accumulation groups"""
            op = ps_sm.tile([128, 64], f32, tag="sm")
            for h in range(2):
                for half in range(2):
                    hc = 128 * half + 64 * h
                    nc.tensor.matmul(
                        op[64 * h:64 * h + 64, :],
                        ex[:, hc:hc + 64],
                        v[:, hc:hc + 64],
                        start=(half == 0), stop=(half == 1),
                        tile_position=(0, 64 * h),
                    )
            return op

        # ---- pipelined compute, emitted in expected execution order ----
        # ---- a-side kproj first: its chunks lead both queues ----
        ks_a = kproj(O_WKA, O_XA, 0, nc.vector.tensor_copy)

        # ---- q projection: qT [128(2h,64ch), 64q] ----
        q_ps = ps_sm.tile([128, 64], f32, tag="sm")
        for k in range(4):
            nc.tensor.matmul(
                q_ps, col(O_WQ + 128 * k, 128), col(O_XMM + 64 * k, 64),
                start=(k == 0), stop=(k == 3),
            )
        # q2z [128(h,ch), 128(h,q)] block-diagonal: head h's scaled q block at
        # [64h:64h+64, 64h:64h+64].  This lets each scores tile be ONE full
        # 128-contract matmul (the off-head contributions multiply by zero)
        # instead of per-head row-group matmuls (which hang the PE).
        q2z = work.tile([128, 128], bf16, tag="q2z")
        nc.vector.memset(q2z, 0.0)
        for h in range(2):
            hs = slice(64 * h, 64 * h + 64)
            nc.vector.tensor_scalar_mul(q2z[hs, 64 * h:64 * h + 64], q_ps[hs, :], SCALE)

        exp_a = scores(ks_a, 0)
        v_a = vproj(O_WVA, O_XA, 0, nc.vector)
        ks_v = kproj(O_WKV, O_XV, 1, nc.vector.tensor_copy)
        exp_v = scores(ks_v, 1)
        v_v = vproj(O_WVV, O_XV, 1, nc.vector)
        zr_a = zsum(exp_a, 0)
        o_a = pv(exp_a, v_a, 0)
        zr_v = zsum(exp_v, 1)
        o_v = pv(exp_v, v_v, 1)

        # ---- per-head: normalize both sides, add, transpose, assemble.
        # The transpose of head h's [64(h,q), 64ch] block is placed at PSUM
        # partitions 64h so the oA copies stay partition-aligned.  The a-side
        # muls are hoisted so they run as soon as o_a/zr_a exist. ----
        oA = work.tile([128, 64], bf16, tag="oA")
        on_a = work.tile([128, 64], f32, tag="on_a")
        on_v = work.tile([128, 64], f32, tag="on_v")
        oc = work.tile([128, 64], bf16, tag="oc")
        oT_ps = ps_f.tile([128, 64], bf16, tag="f")
        nc.vector.tensor_scalar_mul(on_a, o_a, zr_a)
        nc.vector.tensor_scalar_mul(on_v, o_v, zr_v)
        for h in range(2):
            hs = slice(64 * h, 64 * h + 64)
            nc.vector.tensor_add(oc[hs, :], on_a[hs, :], on_v[hs, :])
            nc.tensor.transpose(
                oT_ps[hs, :], oc[hs, :], ident[hs, hs],
                tile_position=(64 * h, 64 * h),
            )
            (nc.scalar.copy if h == 0 else nc.vector.tensor_copy)(
                oA[hs, :], oT_ps[hs, :]
            )

        # ---- output projection partial: [64q, 512], halves in separate
        # PSUM banks so half 1's matmul doesn't wait on half 0's copy ----
        f_sb = work.tile([64, 512], bf16, tag="f_sb")
        for half in range(2):
            cs = slice(256 * half, 256 * half + 256)
            f_ps = ps_f.tile([64, 256], f32, tag="f")
            nc.tensor.matmul(
                f_ps, oA, col(O_WPROJ + 256 * half, 256),
                start=True, stop=True,
            )
            (nc.vector.tensor_copy if half == 0 else nc.scalar.copy)(
                f_sb[:, cs], f_ps
            )
            getattr(nc, "sync" if half == 0 else "scalar").dma_start(
                out=out_d[:, cs], in_=f_sb[:, cs]
            )

    nc.finalize()
    return nc


def _km(a):
    """[512, C] K-major -> [128, 4*C] (4 k-tiles side by side)."""
    c = a.shape[1]
    return a.reshape(4, 128, c).transpose(1, 0, 2).reshape(128, 4 * c)


def _shard_inputs(xmm, xa, xv, Wq, Wkv, Wproj):
    """Build the 8 per-core input maps (one packed [128, 5376] bf16 each)."""
    import ml_dtypes

    in_maps = []
    for core in range(N_CORES):
        b, j = divmod(core, 4)
        r = slice(128 * j, 128 * j + 128)               # head-pair rows in [0,512)
        rv = slice(512 + 128 * j, 512 + 128 * j + 128)  # v rows in Wkv
        pack = np.concatenate(
            [
                _km(Wq[r, :].T),            # O_WQ
                _km(xmm[b].T),              # O_XMM
                _km(Wkv[r, 512:].T),        # O_WKA
                _km(xa[b].T),               # O_XA
                _km(Wkv[rv, 512:].T),       # O_WVA
                _km(Wkv[r, :512].T),        # O_WKV
                _km(xv[b].T),               # O_XV
                _km(Wkv[rv, :512].T),       # O_WVV
                Wproj[:, 128 * j:128 * j + 128].T,  # O_WPROJ
            ],
            axis=1,
        )
        assert pack.shape == (128, PACK_COLS)
        in_maps.append(
            {"packA": np.ascontiguousarray(pack).astype(ml_dtypes.bfloat16)}
        )
    return in_maps


def _get_program():
    if "nc" not in _cached:
        _cached["nc"] = _build_program()
    return _cached["nc"]


def _register_ntff_hook():
    """Best-effort: register the axon NTFF profile hook that the container's
    antenv stub doesn't provide, so run_bass_kernel_spmd(trace=True) can
    measure HW exec time. No-op on failure."""
    try:
        import types

        try:
            from antenv.axon_hooks import get_axon_ntff_profile_hook
            if get_axon_ntff_profile_hook() is not None:
                return
        except ImportError:
            pass
        import antenv
        from trn_agent_boot.trn_boot import _ntff_profile_via_ctypes

        hook = _ntff_profile_via_ctypes("/opt/axon/libaxon_pjrt.so")
        mod = types.ModuleType("antenv.axon_hooks")
        mod._hook = hook
        mod.set_axon_ntff_profile_hook = lambda h: setattr(mod, "_hook", h)
        mod.get_axon_ntff_profile_hook = lambda: mod._hook
        sys.modules["antenv.axon_hooks"] = mod
        antenv.axon_hooks = mod

        # artifact upload has no backing store in this container
        from concourse import bass_utils

        bass_utils.upload_artifacts = lambda tmpdir: tmpdir
    except Exception as e:  # pragma: no cover
        print(f"ntff hook registration failed: {e}", file=sys.stderr)


def kernel(xmm, xa, xv, Wq, Wkv, Wproj, bproj, _want_profile=False):
    from concourse.bass_utils import run_bass_kernel_spmd

    if _want_profile:
        _register_ntff_hook()
    nc = _get_program()
    in_maps = _shard_inputs(
        np.asarray(xmm, np.float32), np.asarray(xa, np.float32),
        np.asarray(xv, np.float32), np.asarray(Wq, np.float32),
        np.asarray(Wkv, np.float32), np.asarray(Wproj, np.float32),
    )
    res = run_bass_kernel_spmd(
        nc, in_maps, core_ids=list(range(N_CORES)), trace=_want_profile
    )
    out = np.zeros((B, N_MM, DIM), np.float32)
    for core in range(N_CORES):
        out[core // 4] += np.asarray(res.results[core]["out"], np.float32)
    out += np.asarray(bproj, np.float32)[None, None, :]
    if _want_profile:
        return out, res
    return out
